# revision 44
# baseline (speedup 1.0000x reference)
"""Trainium2 Bass kernel for Jaccard cosine-similarity edge masking.

out[e] = edge_weight[e] * (sim(e) >= 0.01) * (1 + (src==dst)),
sim(e) = <f_src, f_dst> / (||f_src|| * ||f_dst|| + 1e-8)

Distribution: edges sharded across 8 NeuronCores (symmetric-duplicate
halving when the edge list is the mirrored [[s,d],[d,s]] form with tied
weights). Per-edge feature rows are host-gathered (pure indexing/layout;
descriptor-based device gathers are broken in this toolchain) and streamed
in fp16, d-major interleaved, so every DMA descriptor is a multi-KB
contiguous chunk, round-robined over all three DMA-issuing engines
(SP/ACT hwdge + Pool swdge) to use multiple DMA queues in parallel.

Three NEFFs per call:
  norm: per-node fp32 norms of the row-sharded feature table
        (DVE mul + reduce + ACT sqrt -- bit-matches the numpy reference).
  main: fp16 per-edge inner products. DVE forms fs*fd (fp16 2x mode); the
        PE reduces each 128-edge block via a ones-matmul into PSUM columns
        (fp32 accumulate); keep = inner >= THR*(ns*nd+EPS) plus a border-
        line flag |inner - rhs| < MARGIN*ns*nd. MARGIN exceeds the worst-
        case fp16 rounding bound (see MARGIN comment), so every unflagged
        decision provably matches the fp32 reference.
  fix:  exact fp32 recompute of the ~1% flagged edges, same op structure
        as the (bit-matching) fp32 baseline path.

Weights stay fp32 end to end, so output VALUES are exact; only the keep
decision needs the precision argument above.
"""

import numpy as np
from contextlib import ExitStack

import concourse.bass as bass
import concourse.tile as tile
from concourse import bacc, mybir
from concourse.bass_utils import run_bass_kernel_spmd

N_NODES = 100000
N_EDGES = 1600000
D = 128
P = 128
N_CORES = 8
THRESHOLD = 0.01
EPS = 1e-8
# Worst-case fp16 pipeline error bound (Cauchy-Schwarz, relative to ns*nd):
# two input roundings + one product rounding (3 * 2^-11 = 1.465e-3) plus
# PE fp32 accumulation (~1e-5) and streamed-norm wobble (~1e-6). 1.6e-3
# strictly dominates it, so every unflagged keep decision is provably
# identical to the fp32 reference decision.
MARGIN = 1.6e-3

M = 40                    # 128-column blocks per load group (main NEFF)
EPG = M * P               # 5120 edges per group
CHUNK_G = 12              # groups per PSUM chunk (<=480 cols, 1 bank)

NODES_PER_CORE = N_NODES // N_CORES          # 12500
NTILES = (NODES_PER_CORE + P - 1) // P       # 98
NODES_PAD = NTILES * P                       # 12544 (zero-padded tail)

FIX_CAP = 2048            # fp32 fixup capacity per core per round
FIX_M = 4                 # tiles per fixup load group (finer pipeline)

_cache = {}


def _geom(epc):
    t = (epc + P - 1) // P
    t = (t + M - 1) // M * M
    return t, t * P, t // M


def _build_norm_nc():
    """Per-core fp32 norms over a d-major (transposed) 12544-col feature
    shard: DVE squares, PE ones-matmul column reductions, ACT sqrt."""
    f32 = mybir.dt.float32
    NSPLIT = 14
    COLS = NODES_PAD // NSPLIT               # 896 (7 tiles per split)
    nc = bacc.Bacc("TRN2", target_bir_lowering=False, debug=False,
                   num_devices=N_CORES)
    feat = nc.dram_tensor("feat_t", [D, NODES_PAD], f32,
                          kind="ExternalInput")
    norm_out = nc.dram_tensor("norm98", [P, NTILES], f32,
                              kind="ExternalOutput")
    with tile.TileContext(nc) as tc, ExitStack() as ctx:
        loads = ctx.enter_context(tc.tile_pool(name="loads", bufs=4))
        scr = ctx.enter_context(tc.tile_pool(name="scr", bufs=3))
        acc = ctx.enter_context(tc.tile_pool(name="acc", bufs=1))
        psum = ctx.enter_context(tc.tile_pool(name="ps", bufs=1,
                                              space="PSUM"))
        ones = acc.tile([P, 1], f32)
        nc.vector.memset(ones[:], 1.0)
        ps = psum.tile([P, NTILES], f32)
        for i in range(NSPLIT):
            c0 = i * COLS
            x = loads.tile([P, COLS], f32, tag="x")
            eng = (nc.sync, nc.scalar, nc.gpsimd)[i % 3]
            eng.dma_start(out=x[:], in_=feat.ap()[:, c0:c0 + COLS])
            sq = scr.tile([P, COLS], f32, tag="sq")
            # a few squares go to Pool to shorten the DVE chain
            seng = nc.gpsimd if i in (4, 8, 12) else nc.vector
            seng.tensor_tensor(out=sq[:], in0=x[:], in1=x[:],
                               op=mybir.AluOpType.mult)
            for t in range(COLS // P):
                col = i * (COLS // P) + t
                nc.tensor.matmul(out=ps[:, col:col + 1],
                                 lhsT=sq[:, t * P:(t + 1) * P],
                                 rhs=ones[:], start=True, stop=True)
        nrm = acc.tile([P, NTILES], f32)
        nc.scalar.sqrt(out=nrm[:], in_=ps[:])
        nc.sync.dma_start(out=norm_out.ap(), in_=nrm[:])
    nc.compile()
    return nc


def _build_main_nc(epc, with_eq):
    """fp16 streaming pass: per-edge inner via PE ones-matmul reductions,
    threshold mask against streamed fp32 norms, borderline flags.
    with_eq=False is the fast path for edge lists with no self-loops
    (diag factor identically 1): src/dst streams and eq math drop out."""
    T, SLOTS, G = _geom(epc)
    f32, f16, i32 = mybir.dt.float32, mybir.dt.float16, mybir.dt.int32
    nc = bacc.Bacc("TRN2", target_bir_lowering=False, debug=False,
                   num_devices=N_CORES)
    fsd = nc.dram_tensor("fsd", [G, D, 2, EPG], f16, kind="ExternalInput")
    w_m = nc.dram_tensor("w_m", [P, T], f32, kind="ExternalInput")
    ns_m = nc.dram_tensor("ns_m", [P, T], f32, kind="ExternalInput")
    nd_m = nc.dram_tensor("nd_m", [P, T], f32, kind="ExternalInput")
    if with_eq:
        src_m = nc.dram_tensor("src_m", [P, T], i32, kind="ExternalInput")
        dst_m = nc.dram_tensor("dst_m", [P, T], i32, kind="ExternalInput")
    wout = nc.dram_tensor("wout", [P, T], f32, kind="ExternalOutput")
    flags = nc.dram_tensor("flags", [P, T], f16, kind="ExternalOutput")

    with tile.TileContext(nc) as tc, ExitStack() as ctx:
        mats = ctx.enter_context(tc.tile_pool(name="mats", bufs=1))
        # larger T (non-symmetric fallback) needs the SBUF the deeper
        # prefetch would otherwise take
        loads = ctx.enter_context(
            tc.tile_pool(name="loads", bufs=5 if T <= 1000 else 4))
        prods = ctx.enter_context(tc.tile_pool(name="prods", bufs=3))
        scr = ctx.enter_context(tc.tile_pool(name="scr", bufs=2))
        psum = ctx.enter_context(tc.tile_pool(name="ps", bufs=2, space="PSUM"))

        w_s = mats.tile([P, T], f32)
        ns_s = mats.tile([P, T], f32)
        nd_s = mats.tile([P, T], f32)
        if with_eq:
            src_s = mats.tile([P, T], i32)
            dst_s = mats.tile([P, T], i32)
        wo_s = mats.tile([P, T], f32)
        fl_s = mats.tile([P, T], f16)
        wd_s = mats.tile([P, T], f32)      # w * diag (diag=1 fast path)
        rhs_s = mats.tile([P, T], f32)
        mg_s = mats.tile([P, T], f32)
        ones = mats.tile([P, 1], f16)
        nc.vector.memset(ones[:], 1.0)

        # fsd group engine schedule: g0 in quarters and g2 in halves across
        # sync+scalar for a fast ramp, g1 on the free Pool queue; the rest
        # round-robin with equal shares.  Pool also runs the offloaded mask
        # ALU; small streams sit mid-queue, before the first chunk's mask.
        RR = (nc.sync, nc.scalar, nc.gpsimd)

        def eng_for(g):
            if g in (1, 4):
                return nc.gpsimd
            if g == 3:
                return nc.sync
            if g == 5:
                return nc.scalar
            return RR[g % 3]

        smalls = {6: [(w_s, w_m)],
                  7: [(ns_s, ns_m), (nd_s, nd_m)]}
        if with_eq:
            smalls[9] = [(src_s, src_m)]
            smalls[10] = [(dst_s, dst_m)]

        chunks = [CHUNK_G] * (G // CHUNK_G)
        if G % CHUNK_G:
            chunks.append(G % CHUNK_G)
        if chunks[-1] > 1:            # keep the tail chunk tiny: short tail
            chunks[-1] -= 1
            chunks.append(1)
        chunk_bounds = []
        gA = 0
        for ch in chunks:
            chunk_bounds.append((gA, gA + ch))
            gA += ch
        assert gA == G, (gA, G)

        def issue_group_load(g):
            x = loads.tile([P, 2, EPG], f16, tag="x")
            if g == 0:
                q = EPG // 4
                for i, eng in enumerate((nc.sync, nc.scalar,
                                         nc.sync, nc.scalar)):
                    eng.dma_start(out=x[:, :, i * q:(i + 1) * q],
                                  in_=fsd.ap()[g, :, :, i * q:(i + 1) * q])
            elif g == 2:
                h = EPG // 2
                nc.sync.dma_start(out=x[:, :, :h],
                                  in_=fsd.ap()[g, :, :, :h])
                nc.scalar.dma_start(out=x[:, :, h:],
                                    in_=fsd.ap()[g, :, :, h:])
            else:
                eng_for(g).dma_start(out=x[:], in_=fsd.ap()[g])
            for (tile_, dram) in smalls.pop(g, ()):
                eng_for(g).dma_start(out=tile_[:], in_=dram.ap())
            return x

        # emit every group load first: each engine's queue then pumps
        # continuously; compute trails by data dependency (the 6-buf
        # loads pool bounds the prefetch depth)
        xs = [issue_group_load(g) for g in range(G)]

        hoist_done = False
        for ci, (gA, gB) in enumerate(chunk_bounds):
            CT = (gB - gA) * M
            c0 = gA * M
            sl = slice(c0, c0 + CT)
            ps_in = psum.tile([P, CT], f32, tag="ps_in")
            for g in range(gA, gB):
                x = xs[g]
                # first and last group run in quarter slices so the ramp
                # (resp. tail) overlaps multiply with transfer (resp. mask)
                nq = 4 if g in (0, G - 1) else 1
                QM = M // nq
                for q in range(nq):
                    qsl = slice(q * QM * P, (q + 1) * QM * P)
                    p_sd = prods.tile([P, QM * P], f16, tag="psd")
                    nc.vector.tensor_mul(out=p_sd[:], in0=x[:, 0, qsl],
                                         in1=x[:, 1, qsl])
                    for m in range(QM):
                        col = (g - gA) * M + q * QM + m
                        nc.tensor.matmul(out=ps_in[:, col:col + 1],
                                         lhsT=p_sd[:, m * P:(m + 1) * P],
                                         rhs=ones[:], start=True, stop=True)
                if g == 11 and not hoist_done:
                    # full-T factors hoisted off the per-chunk critical
                    # path: nsnd/eq on DVE; squared margin, rhs and the
                    # diag weight product on the Pool engine (neuronxcc
                    # Pool codegen allows TT mult/sub + TS mult/add/is_ge
                    # only, so the flag test compares squares)
                    hoist_done = True
                    nc.vector.tensor_mul(out=rhs_s[:], in0=ns_s[:],
                                         in1=nd_s[:])
                    nc.gpsimd.tensor_scalar(out=mg_s[:], in0=rhs_s[:],
                                            scalar1=float(MARGIN),
                                            scalar2=None,
                                            op0=mybir.AluOpType.mult)
                    nc.gpsimd.tensor_tensor(out=mg_s[:], in0=mg_s[:],
                                            in1=mg_s[:],
                                            op=mybir.AluOpType.mult)
                    nc.gpsimd.tensor_scalar(out=rhs_s[:], in0=rhs_s[:],
                                            scalar1=float(EPS),
                                            scalar2=float(THRESHOLD),
                                            op0=mybir.AluOpType.add,
                                            op1=mybir.AluOpType.mult)
                    if with_eq:
                        nc.vector.tensor_tensor(out=wd_s[:], in0=src_s[:],
                                                in1=dst_s[:],
                                                op=mybir.AluOpType.is_equal)
                        nc.vector.tensor_scalar(out=wd_s[:], in0=wd_s[:],
                                                scalar1=1.0, scalar2=None,
                                                op0=mybir.AluOpType.add)
                        nc.gpsimd.tensor_tensor(out=wd_s[:], in0=w_s[:],
                                                in1=wd_s[:],
                                                op=mybir.AluOpType.mult)
            # mask math for this chunk: DVE touches PSUM (diff) and does
            # the final wout multiply; keep and the squared-margin flag
            # run on Pool.
            A = scr.tile([P, CT], f32, tag="A")
            B = scr.tile([P, CT], f32, tag="B")
            C = scr.tile([P, CT], f32, tag="C")
            nc.vector.tensor_tensor(out=A[:], in0=ps_in[:],
                                    in1=rhs_s[:, sl],
                                    op=mybir.AluOpType.subtract)  # diff
            nc.gpsimd.tensor_scalar(out=B[:], in0=A[:],        # keep
                                    scalar1=0.0, scalar2=None,
                                    op0=mybir.AluOpType.is_ge)
            nc.gpsimd.tensor_tensor(out=C[:], in0=A[:], in1=A[:],
                                    op=mybir.AluOpType.mult)   # diff^2
            nc.gpsimd.tensor_tensor(out=C[:], in0=mg_s[:, sl], in1=C[:],
                                    op=mybir.AluOpType.subtract)
            nc.gpsimd.tensor_scalar(out=fl_s[:, sl], in0=C[:],  # flag
                                    scalar1=0.0, scalar2=None,
                                    op0=mybir.AluOpType.is_ge)
            wd = wd_s[:, sl] if with_eq else w_s[:, sl]
            nc.vector.tensor_mul(out=wo_s[:, sl], in0=B[:], in1=wd)
            nc.sync.dma_start(out=wout.ap()[:, sl], in_=wo_s[:, sl])
            nc.scalar.dma_start(out=flags.ap()[:, sl], in_=fl_s[:, sl])
    nc.compile()
    return nc


def _build_fix_nc():
    """fp32 exact pass over flagged edges; identical op structure to the
    bit-matching fp32 baseline path (streamed norms from the norm NEFF)."""
    C = FIX_CAP
    T3 = C // P
    G3 = T3 // FIX_M
    f32, i32 = mybir.dt.float32, mybir.dt.int32
    nc = bacc.Bacc("TRN2", target_bir_lowering=False, debug=False,
                   num_devices=N_CORES)
    fs_big = nc.dram_tensor("fs_big", [C, D], f32, kind="ExternalInput")
    fd_big = nc.dram_tensor("fd_big", [C, D], f32, kind="ExternalInput")
    w3 = nc.dram_tensor("w3", [P, T3], f32, kind="ExternalInput")
    ns3 = nc.dram_tensor("ns3", [P, T3], f32, kind="ExternalInput")
    nd3 = nc.dram_tensor("nd3", [P, T3], f32, kind="ExternalInput")
    s3 = nc.dram_tensor("s3", [P, T3], i32, kind="ExternalInput")
    d3 = nc.dram_tensor("d3", [P, T3], i32, kind="ExternalInput")
    wout3 = nc.dram_tensor("wout3", [P, T3], f32, kind="ExternalOutput")

    with tile.TileContext(nc) as tc, ExitStack() as ctx:
        mats = ctx.enter_context(tc.tile_pool(name="mats", bufs=1))
        loads = ctx.enter_context(tc.tile_pool(name="loads", bufs=4))
        scr = ctx.enter_context(tc.tile_pool(name="scr", bufs=3))

        w_s = mats.tile([P, T3], f32)
        ns_s = mats.tile([P, T3], f32)
        nd_s = mats.tile([P, T3], f32)
        s_s = mats.tile([P, T3], i32)
        d_s = mats.tile([P, T3], i32)
        inner = mats.tile([P, T3], f32)

        engs = (nc.sync, nc.scalar, nc.gpsimd)
        fsmalls = {0: [(w_s, w3)], 1: [(ns_s, ns3), (nd_s, nd3)],
                   2: [(s_s, s3), (d_s, d3)]}
        for g in range(G3):
            r0 = g * FIX_M * P
            gsl = slice(g * FIX_M, (g + 1) * FIX_M)
            fs = loads.tile([P, FIX_M, D], f32, tag="fs")
            fd = loads.tile([P, FIX_M, D], f32, tag="fd")
            e0 = engs[(2 * g) % 3]
            e1 = engs[(2 * g + 1) % 3]
            e0.dma_start(
                out=fs[:],
                in_=fs_big.ap()[r0:r0 + FIX_M * P, :].rearrange(
                    "(m p) d -> p m d", p=P))
            e1.dma_start(
                out=fd[:],
                in_=fd_big.ap()[r0:r0 + FIX_M * P, :].rearrange(
                    "(m p) d -> p m d", p=P))
            for (tile_, dram) in fsmalls.pop(g, ()):
                engs[(2 * g + 2) % 3].dma_start(out=tile_[:], in_=dram.ap())
            if g == G3 - 1:   # few groups: issue any leftover small streams
                for k in sorted(fsmalls):
                    for (tile_, dram) in fsmalls.pop(k):
                        engs[k % 3].dma_start(out=tile_[:], in_=dram.ap())
            prod = scr.tile([P, FIX_M, D], f32, tag="prod")
            nc.vector.tensor_mul(out=prod[:], in0=fs[:], in1=fd[:])
            nc.vector.tensor_reduce(out=inner[:, gsl], in_=prod[:],
                                    axis=mybir.AxisListType.X,
                                    op=mybir.AluOpType.add)
        q = mats.tile([P, T3], f32)
        keep = mats.tile([P, T3], f32)
        eq = mats.tile([P, T3], f32)
        wo = mats.tile([P, T3], f32)
        nc.vector.tensor_mul(out=q[:], in0=ns_s[:], in1=nd_s[:])
        nc.vector.tensor_scalar(out=q[:], in0=q[:],
                                scalar1=float(EPS), scalar2=float(THRESHOLD),
                                op0=mybir.AluOpType.add,
                                op1=mybir.AluOpType.mult)
        nc.vector.tensor_tensor(out=keep[:], in0=inner[:], in1=q[:],
                                op=mybir.AluOpType.is_ge)
        nc.vector.tensor_tensor(out=eq[:], in0=s_s[:], in1=d_s[:],
                                op=mybir.AluOpType.is_equal)
        nc.vector.tensor_scalar(out=eq[:], in0=eq[:],
                                scalar1=1.0, scalar2=None,
                                op0=mybir.AluOpType.add)
        nc.vector.tensor_mul(out=wo[:], in0=w_s[:], in1=keep[:])
        nc.vector.tensor_mul(out=wo[:], in0=wo[:], in1=eq[:])
        nc.sync.dma_start(out=wout3.ap(), in_=wo[:])
    nc.compile()
    return nc


def _get(name, builder):
    if name not in _cache:
        _cache[name] = builder()
    return _cache[name]


def _to_mat(v, T):
    return np.ascontiguousarray(v.reshape(T, P).T)


def kernel(edge_index, edge_weight, features, _timing=None):
    edge_index = np.asarray(edge_index)
    edge_weight = np.asarray(edge_weight, dtype=np.float32)
    features = np.ascontiguousarray(np.asarray(features, dtype=np.float32))
    assert edge_index.shape == (2, N_EDGES) and features.shape == (N_NODES, D)

    src_all = edge_index[0].astype(np.int64)
    dst_all = edge_index[1].astype(np.int64)

    half = N_EDGES // 2
    symmetric = (
        np.array_equal(src_all[:half], dst_all[half:])
        and np.array_equal(dst_all[:half], src_all[half:])
        and np.array_equal(edge_weight[:half], edge_weight[half:]))
    n_compute = half if symmetric else N_EDGES
    src, dst, w_all = src_all[:n_compute], dst_all[:n_compute], \
        edge_weight[:n_compute]

    # ---- norm NEFF: per-node norms, row-sharded across the 8 cores ----
    nc1 = _get("norm", _build_norm_nc)
    in_maps1 = []
    for k in range(N_CORES):
        ft = np.zeros((D, NODES_PAD), dtype=np.float32)
        ft[:, :NODES_PER_CORE] = \
            features[k * NODES_PER_CORE:(k + 1) * NODES_PER_CORE].T
        in_maps1.append({"feat_t": ft})
    res1 = run_bass_kernel_spmd(nc1, in_maps1, core_ids=list(range(N_CORES)),
                                **(_timing or {}))
    norm_full = np.empty(N_NODES, dtype=np.float32)
    for k in range(N_CORES):
        out98 = res1.results[k]["norm98"]           # [128, 98]
        base = k * NODES_PER_CORE
        norm_full[base:base + NODES_PER_CORE] = \
            out98.T.ravel()[:NODES_PER_CORE]

    # ---- main NEFF: fp16 streaming inner products + mask + flags ----
    epc = n_compute // N_CORES
    T, SLOTS, G = _geom(epc)
    with_eq = bool((src == dst).any())   # control-flow: self-loop path
    nc2 = _get(f"main{epc}_{with_eq}",
               lambda: _build_main_nc(epc, with_eq))

    f16T = np.ascontiguousarray(features.astype(np.float16).T)  # [D, N]

    in_maps = []
    core_s, core_d, core_w = [], [], []
    for k in range(N_CORES):
        lo = k * epc
        s = np.zeros(SLOTS, dtype=np.int64)
        d = np.zeros(SLOTS, dtype=np.int64)
        w = np.zeros(SLOTS, dtype=np.float32)
        s[:epc] = src[lo:lo + epc]
        d[:epc] = dst[lo:lo + epc]
        w[:epc] = w_all[lo:lo + epc]
        core_s.append(s); core_d.append(d); core_w.append(w)
        fsd = np.empty((G, D, 2, EPG), dtype=np.float16)
        fsd[:, :, 0, :] = f16T[:, s.reshape(G, EPG)].transpose(1, 0, 2)
        fsd[:, :, 1, :] = f16T[:, d.reshape(G, EPG)].transpose(1, 0, 2)
        ns = np.zeros(SLOTS, dtype=np.float32)
        nd = np.zeros(SLOTS, dtype=np.float32)
        ns[:epc] = norm_full[s[:epc]]
        nd[:epc] = norm_full[d[:epc]]
        if epc < SLOTS:   # pad slots -> zero rows: never kept, never flagged
            pad = np.arange(epc, SLOTS)
            gi, ci = pad // EPG, pad % EPG
            fsd[gi, :, 0, ci] = 0
            fsd[gi, :, 1, ci] = 0
        im = {
            "fsd": fsd,
            "w_m": _to_mat(w, T),
            "ns_m": _to_mat(ns, T),
            "nd_m": _to_mat(nd, T),
        }
        if with_eq:
            im["src_m"] = _to_mat(s.astype(np.int32), T)
            im["dst_m"] = _to_mat(d.astype(np.int32), T)
        in_maps.append(im)
    res2 = run_bass_kernel_spmd(nc2, in_maps, core_ids=list(range(N_CORES)),
                                **(_timing or {}))

    out = np.empty(N_EDGES, dtype=edge_weight.dtype)
    flag_idx = []
    for k in range(N_CORES):
        wo = res2.results[k]["wout"]            # [P, T]
        fl = res2.results[k]["flags"]
        out[k * epc:(k + 1) * epc] = wo.T.ravel()[:epc]
        flag_idx.append(np.nonzero(fl.T.ravel()[:epc] != 0.0)[0])

    # ---- fix NEFF: exact fp32 recompute of flagged edges ----
    res3_list = []
    if max(len(f) for f in flag_idx) > 0:
        rounds = max((len(f) + FIX_CAP - 1) // FIX_CAP for f in flag_idx)
        nc3 = _get("fix", _build_fix_nc)
        T3 = FIX_CAP // P
        for r in range(rounds):
            in3, idxs = [], []
            for k in range(N_CORES):
                idx = flag_idx[k][r * FIX_CAP:(r + 1) * FIX_CAP]
                idxs.append(idx)
                n = len(idx)
                fs_b = np.zeros((FIX_CAP, D), dtype=np.float32)
                fd_b = np.zeros((FIX_CAP, D), dtype=np.float32)
                w3 = np.zeros(FIX_CAP, dtype=np.float32)
                ns3 = np.zeros(FIX_CAP, dtype=np.float32)
                nd3 = np.zeros(FIX_CAP, dtype=np.float32)
                s3 = np.zeros(FIX_CAP, dtype=np.int32)
                d3 = np.zeros(FIX_CAP, dtype=np.int32)
                if n:
                    sk = core_s[k][idx]; dk = core_d[k][idx]
                    fs_b[:n] = features[sk]
                    fd_b[:n] = features[dk]
                    w3[:n] = core_w[k][idx]
                    ns3[:n] = norm_full[sk]
                    nd3[:n] = norm_full[dk]
                    s3[:n] = sk
                    d3[:n] = dk
                in3.append({
                    "fs_big": fs_b, "fd_big": fd_b,
                    "w3": _to_mat(w3, T3),
                    "ns3": _to_mat(ns3, T3),
                    "nd3": _to_mat(nd3, T3),
                    "s3": _to_mat(s3, T3),
                    "d3": _to_mat(d3, T3),
                })
            res3 = run_bass_kernel_spmd(nc3, in3,
                                        core_ids=list(range(N_CORES)),
                                        **(_timing or {}))
            res3_list.append(res3)
            for k in range(N_CORES):
                idx = idxs[k]
                if len(idx):
                    vals = res3.results[k]["wout3"].T.ravel()[:len(idx)]
                    out[k * epc + idx] = vals

    if symmetric:
        out[half:] = out[:half]
    if _timing is not None:
        kernel._last = (res1, res2, res3_list)
        kernel._ncs = (nc1, nc2, _cache.get("fix"))
        kernel._rounds = len(res3_list)
    return out
